# revision 28
# baseline (speedup 1.0000x reference)
"""Mixtral MoE MLP (ragged grouped-GEMM SwiGLU) on 8 Trainium2 NeuronCores.

Sharding: tensor-parallel over the intermediate dim DI. Core c owns the
DI columns [c*DI/8, (c+1)*DI/8) of w1/w3 (and the matching rows of w2)
for ALL experts. Every core processes ALL tokens -> the Bass program is
identical across cores (true SPMD; only the weight data differs per
core). Each core produces a partial y (rank-DI/8 contribution); the
8-way sum ("all-reduce after w2") is done on the host after gather.

DMA schedule: everything on the SP HWDGE ring (a second ring shares the
8 DMA sem lanes and causes cross-ring FIFO coupling -- measured worse).
Weights are prefetched one full EXPERT ahead (emitted at the first
chunk of the previous expert), x one chunk ahead. PE stalls came from
the chain GEMM2-MM -> cast (psum recycle) -> store (ot-slot WAR) ->
DMA-lane predecessor (a weight burst); broken by chunk-deep ot slack,
OGRP-batched stores, and the expert-ahead prefetch smoothing bursts.

Per-core compute: 3 * 2*NT*DH*(DI/8) = ~25.8 GFLOP, bf16 matmuls with
fp32 PSUM accumulation. Measured ~356-365us/core at 2.4 GHz (PE stream
floor 327.7us); runs land ~1.2x slower when the chip is in the P0
2.0 GHz power state (back-to-back executions).

Layouts (host-packed; x/y chunk-contiguous so every DMA is one
contiguous run per partition):
  xh  [128, KD*NT]   xh[p, KD*t0+k*n+t] = x[t0+t, k*128+p]      (bf16)
  w1h [NE, MD, 128, KD, 128]  w1h[e,mi,p,k,j] = w1[e, k*128+p, dlo+mi*128+j]
  w3h same as w1h
  w2h [NE, GH, 128, MD, 512]  w2h[e,g,p,k,j]  = w2[e, dlo+k*128+p, g*512+j]
  yh  [MO/4, 128, 4*NT]  yh[g,p,4*t0+j*n+t] = y_part[t0+t, (4g+j)*128+p]
where dlo = c*DI_SH is this core's DI offset, (t0, n) a chunk.
"""

import os
import sys

import numpy as np
import ml_dtypes

for _p in ("/opt/trn_rl_repo", "/root/.axon_site/_ro/trn_rl_repo"):
    if os.path.isdir(_p) and _p not in sys.path:
        sys.path.append(_p)

import concourse.bass as bass  # noqa: E402
import concourse.bacc as bacc  # noqa: E402
import concourse.tile as tile  # noqa: E402
import concourse.mybir as mybir  # noqa: E402
from concourse.bass_utils import run_bass_kernel_spmd  # noqa: E402


def _ensure_ntff_hook_shim():
    """concourse's trace path imports antenv.axon_hooks, which this image
    lacks; provide a functional stand-in so tracing works (or degrades
    gracefully) instead of raising ImportError."""
    try:
        import antenv.axon_hooks  # noqa: F401
        return
    except Exception:
        pass
    import types

    try:
        import antenv
    except Exception:
        antenv = types.ModuleType("antenv")
        sys.modules["antenv"] = antenv
    mod = types.ModuleType("antenv.axon_hooks")
    state = {"hook": None, "tried": False}

    def set_axon_ntff_profile_hook(h):
        state["hook"] = h

    def get_axon_ntff_profile_hook():
        if state["hook"] is None and not state["tried"]:
            state["tried"] = True
            try:
                from trn_agent_boot.trn_boot import _ntff_profile_via_ctypes

                state["hook"] = _ntff_profile_via_ctypes(
                    "/opt/axon/libaxon_pjrt.so"
                )
            except Exception:
                state["hook"] = None
        return state["hook"]

    mod.set_axon_ntff_profile_hook = set_axon_ntff_profile_hook
    mod.get_axon_ntff_profile_hook = get_axon_ntff_profile_hook
    sys.modules["antenv.axon_hooks"] = mod
    antenv.axon_hooks = mod


_ensure_ntff_hook_shim()

BF16 = mybir.dt.bfloat16
F32 = mybir.dt.float32
NPBF16 = ml_dtypes.bfloat16

N_CORES = 8
P = 128
NMAX = 512  # max matmul moving free dim (one PSUM bank of fp32)

# Knobs for experimentation from test.py
TRACE = False
TRACE_CORES = None
LAST_RESULTS = None

_prog_cache: dict = {}


def _plan_chunks(group_sizes):
    """Split each expert's token range into near-equal chunks of <= NMAX.

    Returns (chunks, nt_eff) where chunks is a list of (expert, t0, n).
    Chunks are emitted grouped by expert, in a schedule order chosen so
    that (a) the first expert is small (short time-to-first-matmul),
    (b) each expert's weight load is covered by a preceding large
    compute window, (c) the last chunk is small (short output drain).
    """
    per_e = {}
    off = 0
    for e, g in enumerate(group_sizes):
        g = int(g)
        if g > 0:
            k = -(-g // NMAX)
            base, rem = divmod(g, k)
            t = off
            cl = []
            for i in range(k):
                n = base + (1 if i < rem else 0)
                cl.append((e, t, n))
                t += n
            per_e[e] = cl
        off += g

    # Biggest experts first: their long compute windows cover successor
    # weight loads; the expert with the smallest final chunk goes last
    # (short output drain before the exit barrier).
    order = sorted(per_e, key=lambda e: -sum(c[2] for c in per_e[e]))
    if len(order) > 1:
        last = min(order, key=lambda e: per_e[e][-1][2])
        order.remove(last)
        order.append(last)

    chunks = [c for e in order for c in per_e[e]]
    return chunks, off


W2G = 512  # w2 blocks carry this many DH columns per DMA
OGRP = 4  # output m-tiles batched per store DMA (sync-engine issue rate)


def _build_program(ne, dh, di_sh, nt, chunks):
    kd = dh // P     # k-tiles for gemm1/3 (contraction over DH)
    md = di_sh // P  # m-tiles for gemm1/3 == k-tiles for gemm2
    mo = dh // P     # m-tiles for gemm2 (output DH)
    gh = dh // W2G   # w2 DMA groups (W2G//P m-tiles per group)
    gm = W2G // P

    nc = bacc.Bacc(
        "TRN2", target_bir_lowering=False, debug=False, num_devices=N_CORES
    )
    # x and y are packed CHUNK-CONTIGUOUS on the host (chunk ci's block
    # starts at column kd*t0 / OGRP*t0): each chunk DMA is then a single
    # contiguous run per partition (16 KB) instead of kd strided ~1 KB
    # runs -> ~16x fewer descriptors, faster + lower-variance startup.
    xh = nc.dram_tensor("xh", [P, kd * nt], BF16, kind="ExternalInput")
    w1h = nc.dram_tensor("w1h", [ne, md, P, kd, P], BF16, kind="ExternalInput")
    w3h = nc.dram_tensor("w3h", [ne, md, P, kd, P], BF16, kind="ExternalInput")
    w2h = nc.dram_tensor("w2h", [ne, gh, P, md, W2G], BF16, kind="ExternalInput")
    yh = nc.dram_tensor("yh", [mo // OGRP, P, OGRP * nt], BF16, kind="ExternalOutput")

    silu = mybir.ActivationFunctionType.Silu

    # Expert schedule: first chunk index of each expert, in chunk order.
    expert_seq = []  # (first_chunk_idx, expert)
    for ci, (e, t0, n) in enumerate(chunks):
        if ci == 0 or chunks[ci - 1][0] != e:
            expert_seq.append((ci, e))
    # At the first chunk of expert k, prefetch weights of expert k+1 —
    # a full expert of DMA lead (>=24us) instead of one chunk.
    next_expert_at = {
        ci: expert_seq[k + 1][1]
        for k, (ci, e) in enumerate(expert_seq)
        if k + 1 < len(expert_seq)
    }

    with tile.TileContext(nc) as tc:
        with (
            tc.tile_pool(name="w1pool", bufs=12) as w1pool,
            tc.tile_pool(name="w3pool", bufs=12) as w3pool,
            tc.tile_pool(name="w2pool", bufs=8) as w2pool,
            tc.tile_pool(name="xpool", bufs=2) as xpool,
            tc.tile_pool(name="hpool", bufs=2) as hpool,
            tc.tile_pool(name="cpool", bufs=4) as cpool,
            tc.tile_pool(name="psh", bufs=2, space="PSUM") as psh,
            tc.tile_pool(name="psy", bufs=4, space="PSUM") as psy,
        ):
            # PE pre-warm: dummy matmuls on a DVE-zeroed scratch tile (DVE
            # memset clears ~3.5us after engine init vs ~7us for the
            # gpsimd path) keep the PE-busy window unbroken until the
            # first real matmul's inputs land, so the HAM clock-gate
            # reaches K=8/8 before real work.
            # 120 warmups bridge the PE from ~7.3us (engine init done)
            # until the first chunk's x lands (~16us; sync-engine DMA
            # issue rate + early-DMA latency bound) -- HAM flips to 8/8
            # at ~10.7us, so the real stream starts warm and gapless.
            warm_sb = cpool.tile([P, P], BF16, tag="warm", bufs=1)
            nc.vector.memset(warm_sb[:], 0.0)
            warm_ps = psy.tile([P, P], F32, tag="y", name="warm_ps")
            for _ in range(125):
                nc.tensor.matmul(warm_ps[:], warm_sb[:], warm_sb[:])

            wtiles = {}

            def emit_weights(e, crit_x=None):
                w1b = [
                    w1pool.tile([P, kd, P], BF16, tag="w1", name=f"w1b{e}_{i}")
                    for i in range(md)
                ]
                w3b = [
                    w3pool.tile([P, kd, P], BF16, tag="w3", name=f"w3b{e}_{i}")
                    for i in range(md)
                ]
                w2b = [
                    w2pool.tile([P, md, W2G], BF16, tag="w2", name=f"w2b{e}_{i}")
                    for i in range(gh)
                ]
                wtiles[e] = (w1b, w3b, w2b)
                if crit_x is not None:
                    # kernel-entry critical path: interleave w1 block 0, x
                    # and w3 block 0 in k-halves. w3b0 must land with the
                    # early pieces: GEMM3-m0 starts ~2.4us after the first
                    # real matmul and a late w3b0 stalls the PE (measured
                    # 2.5us + a HAM re-throttle on slow cores).
                    x_sb, t0, n = crit_x
                    kh = kd // 2
                    for q in range(2):
                        sl = slice(q * kh, (q + 1) * kh)
                        csl = slice(kd * t0 + q * kh * n, kd * t0 + (q + 1) * kh * n)
                        nc.sync.dma_start(w1b[0][:, sl, :], w1h[e, 0, :, sl])
                        nc.sync.dma_start(x_sb[:, q * kh * n : (q + 1) * kh * n], xh[:, csl])
                        nc.sync.dma_start(w3b[0][:, sl, :], w3h[e, 0, :, sl])
                else:
                    nc.sync.dma_start(w1b[0][:], w1h[e, 0])
                    nc.sync.dma_start(w3b[0][:], w3h[e, 0])
                for mi in range(1, md):
                    nc.sync.dma_start(w1b[mi][:], w1h[e, mi])
                    nc.sync.dma_start(w3b[mi][:], w3h[e, mi])
                for g in range(gh):
                    nc.sync.dma_start(w2b[g][:], w2h[e, g])

            def emit_x(ci):
                e, t0, n = chunks[ci]
                x_sb = xpool.tile([P, kd * n], BF16, tag="x", name=f"x{ci}")
                nc.sync.dma_start(x_sb[:], xh[:, kd * t0 : kd * (t0 + n)])
                return x_sb

            # Kernel entry: expert 0's weights + chunk 0's x.
            e0, t00, n0 = chunks[0]
            x0 = xpool.tile([P, kd * n0], BF16, tag="x", name="x0")
            emit_weights(e0, crit_x=(x0, t00, n0))
            x_pending = {0: x0}

            for ci, (e, t0, n) in enumerate(chunks):
                if ci + 1 < len(chunks):
                    x_pending[ci + 1] = emit_x(ci + 1)
                if ci in next_expert_at:
                    emit_weights(next_expert_at[ci])
                x_sb = x_pending.pop(ci)
                w1b, w3b, w2b = wtiles[e]

                h_sb = hpool.tile([P, md, n], BF16, tag="h")
                for mi in range(md):
                    ps1 = psh.tile([P, n], F32, tag="h1")
                    for k in range(kd):
                        nc.tensor.matmul(
                            ps1[:],
                            w1b[mi][:, k, :],
                            x_sb[:, k * n : (k + 1) * n],
                            start=(k == 0),
                            stop=(k == kd - 1),
                        )
                    ps3 = psh.tile([P, n], F32, tag="h3")
                    for k in range(kd):
                        nc.tensor.matmul(
                            ps3[:],
                            w3b[mi][:, k, :],
                            x_sb[:, k * n : (k + 1) * n],
                            start=(k == 0),
                            stop=(k == kd - 1),
                        )
                    # silu(h1) * h3 via the HW Silu LUT (one ACT + one mul;
                    # accuracy verified identical to sigmoid+muls on HW)
                    sl = cpool.tile([P, n], F32, tag="silu", bufs=2)
                    nc.scalar.activation(sl[:], ps1[:], silu)
                    nc.vector.tensor_mul(h_sb[:, mi, :], sl[:], ps3[:])

                # GEMM2 with split-k emission: the first 3 m-tiles run
                # k=0..md-2 before anything touches h k-tile md-1 (whose
                # SwiGLU finishes last), hiding the ACT/DVE latency tail.
                NPRE = 3
                pre = []
                for m in range(min(NPRE, mo)):
                    psy_t = psy.tile([P, n], F32, tag="y", name=f"ypre{m}")
                    for k in range(md - 1):
                        nc.tensor.matmul(
                            psy_t[:],
                            w2b[m // gm][:, k, (m % gm) * P : (m % gm + 1) * P],
                            h_sb[:, k, :],
                            start=(k == 0),
                            stop=False,
                        )
                    pre.append(psy_t)
                ot4 = None
                for m in range(mo):
                    if m < len(pre):
                        psy_t = pre[m]
                        nc.tensor.matmul(
                            psy_t[:],
                            w2b[m // gm][:, md - 1, (m % gm) * P : (m % gm + 1) * P],
                            h_sb[:, md - 1, :],
                            start=False,
                            stop=True,
                        )
                    else:
                        psy_t = psy.tile([P, n], F32, tag="y")
                        for k in range(md):
                            nc.tensor.matmul(
                                psy_t[:],
                                w2b[m // gm][:, k, (m % gm) * P : (m % gm + 1) * P],
                                h_sb[:, k, :],
                                start=(k == 0),
                                stop=(k == md - 1),
                            )
                    # OGRP m-tiles share one ot tile and one store DMA:
                    # fewer stores keeps the sync engine's ~0.65us/issue
                    # rate off the critical path (drains the kernel tail),
                    # and 4 bufs = a full chunk of cast->store slack.
                    if m % OGRP == 0:
                        ot4 = cpool.tile([P, OGRP * n], BF16, tag="o", bufs=4)
                    j = m % OGRP
                    nc.vector.tensor_copy(ot4[:, j * n : (j + 1) * n], psy_t[:])
                    if m % OGRP == OGRP - 1:
                        nc.sync.dma_start(
                            yh[m // OGRP, :, OGRP * t0 : OGRP * (t0 + n)], ot4[:]
                        )

    nc.compile()
    return nc


def _get_program(ne, dh, di, nt_eff, chunk_key):
    key = (ne, dh, di, nt_eff, chunk_key)
    prog = _prog_cache.get(key)
    if prog is None:
        chunks = [tuple(c) for c in chunk_key]
        prog = _build_program(ne, dh, di // N_CORES, nt_eff, chunks)
        _prog_cache[key] = prog
    return prog


def kernel(x, group_sizes, w1, w2, w3):
    x = np.asarray(x, dtype=np.float32)
    group_sizes = np.asarray(group_sizes)
    w1 = np.asarray(w1, dtype=np.float32)
    w2 = np.asarray(w2, dtype=np.float32)
    w3 = np.asarray(w3, dtype=np.float32)

    ne, dh, di = w1.shape
    di_sh = di // N_CORES
    kd = dh // P
    md = di_sh // P

    chunks, nt_eff = _plan_chunks(group_sizes)
    if nt_eff == 0:
        return np.zeros((0, dh), dtype=np.float32)

    nc = _get_program(ne, dh, di, nt_eff, tuple(chunks))

    # ---- host-side pack / shard ----
    xe = x[:nt_eff]
    # chunk-contiguous x: chunk at column kd*t0 holds [P, kd, n] C-order,
    # i.e. xh[p, kd*t0 + k*n + t] = x[t0+t, k*128+p]
    xh = np.empty((P, kd * nt_eff), dtype=NPBF16)
    for _, t0, n in chunks:
        blk = xe[t0 : t0 + n].astype(NPBF16).reshape(n, kd, P).transpose(2, 1, 0)
        xh[:, kd * t0 : kd * (t0 + n)] = blk.reshape(P, kd * n)
    # w1/w3: [NC, NE, MD, P(part), KD, P(col)]
    # dh = k*128+p ; di = c*di_sh + mi*128 + j
    w1a = np.ascontiguousarray(
        w1.astype(NPBF16)
        .reshape(ne, kd, P, N_CORES, md, P)
        .transpose(3, 0, 4, 2, 1, 5)
    )
    w3a = np.ascontiguousarray(
        w3.astype(NPBF16)
        .reshape(ne, kd, P, N_CORES, md, P)
        .transpose(3, 0, 4, 2, 1, 5)
    )
    # w2: [NC, NE, GH, P(part), MD, W2G]
    # di = c*di_sh + k*128 + p ; dh = g*W2G + j
    gh = dh // W2G
    w2a = np.ascontiguousarray(
        w2.astype(NPBF16)
        .reshape(ne, N_CORES, md, P, gh, W2G)
        .transpose(1, 0, 4, 3, 2, 5)
    )

    in_maps = [
        {"xh": xh, "w1h": w1a[c], "w3h": w3a[c], "w2h": w2a[c]}
        for c in range(N_CORES)
    ]

    global LAST_RESULTS
    res = run_bass_kernel_spmd(
        nc,
        in_maps,
        core_ids=list(range(N_CORES)),
        trace=TRACE,
        trace_cores=TRACE_CORES,
    )
    LAST_RESULTS = res

    # ---- host-side gather + 8-way partial sum ("all-reduce") ----
    # yh[g, p, OGRP*t0 + j*n + t] = y_partial[t0+t, (OGRP*g+j)*128 + p]
    mo = dh // P
    G = mo // OGRP
    acc = np.zeros((nt_eff, dh), dtype=np.float32)
    for c in range(N_CORES):
        yh = res.results[c]["yh"]  # [G, P, OGRP*nt] bf16
        for _, t0, n in chunks:
            blk = yh[:, :, OGRP * t0 : OGRP * (t0 + n)].astype(np.float32)
            # (g, p, j, t) -> (t, g, j, p)
            blk = blk.reshape(G, P, OGRP, n).transpose(3, 0, 2, 1)
            acc[t0 : t0 + n] += blk.reshape(n, dh)
    return np.ascontiguousarray(acc)



# revision 29
# speedup vs baseline: 1.1966x; 1.1966x over previous
"""Mixtral MoE MLP (ragged grouped-GEMM SwiGLU) on 8 Trainium2 NeuronCores.

Sharding: tensor-parallel over the intermediate dim DI. Core c owns the
DI columns [c*DI/8, (c+1)*DI/8) of w1/w3 (and the matching rows of w2)
for ALL experts. Every core processes ALL tokens -> the Bass program is
identical across cores (true SPMD; only the weight data differs per
core). Each core produces a partial y (rank-DI/8 contribution); the
8-way sum ("all-reduce after w2") is done on the host after gather.

DMA schedule: everything on the SP HWDGE ring (a second ring shares the
8 DMA sem lanes and causes cross-ring FIFO coupling -- measured worse).
Weights are prefetched one full EXPERT ahead (emitted at the first
chunk of the previous expert), x one chunk ahead. PE stalls came from
the chain GEMM2-MM -> cast (psum recycle) -> store (ot-slot WAR) ->
DMA-lane predecessor (a weight burst); broken by chunk-deep ot slack,
OGRP-batched stores, and the expert-ahead prefetch smoothing bursts.

Per-core compute: 3 * 2*NT*DH*(DI/8) = ~25.8 GFLOP, bf16 matmuls with
fp32 PSUM accumulation. Measured ~356-365us/core at 2.4 GHz (PE stream
floor 327.7us); runs land ~1.2x slower when the chip is in the P0
2.0 GHz power state (back-to-back executions).

Layouts (host-packed; x/y chunk-contiguous so every DMA is one
contiguous run per partition):
  xh  [128, KD*NT]   xh[p, KD*t0+k*n+t] = x[t0+t, k*128+p]      (bf16)
  w1h [NE, MD, 128, KD, 128]  w1h[e,mi,p,k,j] = w1[e, k*128+p, dlo+mi*128+j]
  w3h same as w1h
  w2h [NE, GH, 128, MD, 512]  w2h[e,g,p,k,j]  = w2[e, dlo+k*128+p, g*512+j]
  yh  [MO/4, 128, 4*NT]  yh[g,p,4*t0+j*n+t] = y_part[t0+t, (4g+j)*128+p]
where dlo = c*DI_SH is this core's DI offset, (t0, n) a chunk.
"""

import os
import sys

import numpy as np
import ml_dtypes

for _p in ("/opt/trn_rl_repo", "/root/.axon_site/_ro/trn_rl_repo"):
    if os.path.isdir(_p) and _p not in sys.path:
        sys.path.append(_p)

import concourse.bass as bass  # noqa: E402
import concourse.bacc as bacc  # noqa: E402
import concourse.tile as tile  # noqa: E402
import concourse.mybir as mybir  # noqa: E402
from concourse.bass_utils import run_bass_kernel_spmd  # noqa: E402


def _ensure_ntff_hook_shim():
    """concourse's trace path imports antenv.axon_hooks, which this image
    lacks; provide a functional stand-in so tracing works (or degrades
    gracefully) instead of raising ImportError."""
    try:
        import antenv.axon_hooks  # noqa: F401
        return
    except Exception:
        pass
    import types

    try:
        import antenv
    except Exception:
        antenv = types.ModuleType("antenv")
        sys.modules["antenv"] = antenv
    mod = types.ModuleType("antenv.axon_hooks")
    state = {"hook": None, "tried": False}

    def set_axon_ntff_profile_hook(h):
        state["hook"] = h

    def get_axon_ntff_profile_hook():
        if state["hook"] is None and not state["tried"]:
            state["tried"] = True
            try:
                from trn_agent_boot.trn_boot import _ntff_profile_via_ctypes

                state["hook"] = _ntff_profile_via_ctypes(
                    "/opt/axon/libaxon_pjrt.so"
                )
            except Exception:
                state["hook"] = None
        return state["hook"]

    mod.set_axon_ntff_profile_hook = set_axon_ntff_profile_hook
    mod.get_axon_ntff_profile_hook = get_axon_ntff_profile_hook
    sys.modules["antenv.axon_hooks"] = mod
    antenv.axon_hooks = mod


_ensure_ntff_hook_shim()

BF16 = mybir.dt.bfloat16
F32 = mybir.dt.float32
NPBF16 = ml_dtypes.bfloat16

N_CORES = 8
P = 128
NMAX = 512  # max matmul moving free dim (one PSUM bank of fp32)

# Knobs for experimentation from test.py
TRACE = False
TRACE_CORES = None
LAST_RESULTS = None

_prog_cache: dict = {}


def _plan_chunks(group_sizes):
    """Split each expert's token range into near-equal chunks of <= NMAX.

    Returns (chunks, nt_eff) where chunks is a list of (expert, t0, n).
    Chunks are emitted grouped by expert, in a schedule order chosen so
    that (a) the first expert is small (short time-to-first-matmul),
    (b) each expert's weight load is covered by a preceding large
    compute window, (c) the last chunk is small (short output drain).
    """
    per_e = {}
    off = 0
    for e, g in enumerate(group_sizes):
        g = int(g)
        if g > 0:
            k = -(-g // NMAX)
            base, rem = divmod(g, k)
            t = off
            cl = []
            for i in range(k):
                n = base + (1 if i < rem else 0)
                cl.append((e, t, n))
                t += n
            per_e[e] = cl
        off += g

    # Biggest experts first: their long compute windows cover successor
    # weight loads; the expert with the smallest final chunk goes last
    # (short output drain before the exit barrier).
    order = sorted(per_e, key=lambda e: -sum(c[2] for c in per_e[e]))
    if len(order) > 1:
        last = min(order, key=lambda e: per_e[e][-1][2])
        order.remove(last)
        order.append(last)

    chunks = [c for e in order for c in per_e[e]]
    return chunks, off


W2G = 512  # w2 blocks carry this many DH columns per DMA
OGRP = 4  # output m-tiles batched per store DMA (sync-engine issue rate)


def _build_program(ne, dh, di_sh, nt, chunks):
    kd = dh // P     # k-tiles for gemm1/3 (contraction over DH)
    md = di_sh // P  # m-tiles for gemm1/3 == k-tiles for gemm2
    mo = dh // P     # m-tiles for gemm2 (output DH)
    gh = dh // W2G   # w2 DMA groups (W2G//P m-tiles per group)
    gm = W2G // P

    nc = bacc.Bacc(
        "TRN2", target_bir_lowering=False, debug=False, num_devices=N_CORES
    )
    # x and y are packed CHUNK-CONTIGUOUS on the host (chunk ci's block
    # starts at column kd*t0 / OGRP*t0): each chunk DMA is then a single
    # contiguous run per partition (16 KB) instead of kd strided ~1 KB
    # runs -> ~16x fewer descriptors, faster + lower-variance startup.
    xh = nc.dram_tensor("xh", [P, kd * nt], BF16, kind="ExternalInput")
    w1h = nc.dram_tensor("w1h", [ne, md, P, kd, P], BF16, kind="ExternalInput")
    w3h = nc.dram_tensor("w3h", [ne, md, P, kd, P], BF16, kind="ExternalInput")
    w2h = nc.dram_tensor("w2h", [ne, gh, P, md, W2G], BF16, kind="ExternalInput")
    yh = nc.dram_tensor("yh", [mo // OGRP, P, OGRP * nt], BF16, kind="ExternalOutput")

    silu = mybir.ActivationFunctionType.Silu

    # Expert schedule: first chunk index of each expert, in chunk order.
    expert_seq = []  # (first_chunk_idx, expert)
    for ci, (e, t0, n) in enumerate(chunks):
        if ci == 0 or chunks[ci - 1][0] != e:
            expert_seq.append((ci, e))
    # At the first chunk of expert k, prefetch weights of expert k+1 —
    # a full expert of DMA lead (>=24us) instead of one chunk.
    next_expert_at = {
        ci: expert_seq[k + 1][1]
        for k, (ci, e) in enumerate(expert_seq)
        if k + 1 < len(expert_seq)
    }

    with tile.TileContext(nc) as tc:
        with (
            tc.tile_pool(name="w1pool", bufs=12) as w1pool,
            tc.tile_pool(name="w3pool", bufs=12) as w3pool,
            tc.tile_pool(name="w2pool", bufs=8) as w2pool,
            tc.tile_pool(name="xpool", bufs=2) as xpool,
            tc.tile_pool(name="hpool", bufs=2) as hpool,
            tc.tile_pool(name="cpool", bufs=4) as cpool,
            tc.tile_pool(name="psh", bufs=2, space="PSUM") as psh,
            tc.tile_pool(name="psy", bufs=4, space="PSUM") as psy,
        ):
            # PE pre-warm: dummy matmuls on a DVE-zeroed scratch tile (DVE
            # memset clears ~3.5us after engine init vs ~7us for the
            # gpsimd path) keep the PE-busy window unbroken until the
            # first real matmul's inputs land, so the HAM clock-gate
            # reaches K=8/8 before real work.
            # 125 warmups bridge the PE from ~7.3us (engine init done)
            # until the first chunk's x lands (~16us; sync-engine DMA
            # issue rate + early-DMA latency bound) -- HAM flips to 8/8
            # at ~10.7us, so the real stream starts warm and gapless.
            warm_sb = cpool.tile([P, P], BF16, tag="warm", bufs=1)
            nc.vector.memset(warm_sb[:], 0.0)
            warm_ps = psy.tile([P, P], F32, tag="y", name="warm_ps")
            for _ in range(125):
                nc.tensor.matmul(warm_ps[:], warm_sb[:], warm_sb[:])

            wtiles = {}

            def emit_weights(e, crit_x=None):
                w1b = [
                    w1pool.tile([P, kd, P], BF16, tag="w1", name=f"w1b{e}_{i}")
                    for i in range(md)
                ]
                w3b = [
                    w3pool.tile([P, kd, P], BF16, tag="w3", name=f"w3b{e}_{i}")
                    for i in range(md)
                ]
                w2b = [
                    w2pool.tile([P, md, W2G], BF16, tag="w2", name=f"w2b{e}_{i}")
                    for i in range(gh)
                ]
                wtiles[e] = (w1b, w3b, w2b)
                if crit_x is not None:
                    # kernel-entry critical path: interleave w1 block 0, x
                    # and w3 block 0 in k-halves. w3b0 must land with the
                    # early pieces: GEMM3-m0 starts ~2.4us after the first
                    # real matmul and a late w3b0 stalls the PE (measured
                    # 2.5us + a HAM re-throttle on slow cores).
                    x_sb, t0, n = crit_x
                    kh = kd // 2
                    for q in range(2):
                        sl = slice(q * kh, (q + 1) * kh)
                        csl = slice(kd * t0 + q * kh * n, kd * t0 + (q + 1) * kh * n)
                        nc.sync.dma_start(w1b[0][:, sl, :], w1h[e, 0, :, sl])
                        nc.sync.dma_start(x_sb[:, q * kh * n : (q + 1) * kh * n], xh[:, csl])
                        nc.sync.dma_start(w3b[0][:, sl, :], w3h[e, 0, :, sl])
                else:
                    nc.sync.dma_start(w1b[0][:], w1h[e, 0])
                    nc.sync.dma_start(w3b[0][:], w3h[e, 0])
                for mi in range(1, md):
                    nc.sync.dma_start(w1b[mi][:], w1h[e, mi])
                    nc.sync.dma_start(w3b[mi][:], w3h[e, mi])
                for g in range(gh):
                    nc.sync.dma_start(w2b[g][:], w2h[e, g])

            def emit_x(ci):
                e, t0, n = chunks[ci]
                x_sb = xpool.tile([P, kd * n], BF16, tag="x", name=f"x{ci}")
                nc.sync.dma_start(x_sb[:], xh[:, kd * t0 : kd * (t0 + n)])
                return x_sb

            # Kernel entry: expert 0's weights + chunk 0's x.
            e0, t00, n0 = chunks[0]
            x0 = xpool.tile([P, kd * n0], BF16, tag="x", name="x0")
            emit_weights(e0, crit_x=(x0, t00, n0))
            x_pending = {0: x0}

            for ci, (e, t0, n) in enumerate(chunks):
                if ci + 1 < len(chunks):
                    x_pending[ci + 1] = emit_x(ci + 1)
                if ci in next_expert_at:
                    emit_weights(next_expert_at[ci])
                x_sb = x_pending.pop(ci)
                w1b, w3b, w2b = wtiles[e]

                h_sb = hpool.tile([P, md, n], BF16, tag="h")
                for mi in range(md):
                    ps1 = psh.tile([P, n], F32, tag="h1")
                    for k in range(kd):
                        nc.tensor.matmul(
                            ps1[:],
                            w1b[mi][:, k, :],
                            x_sb[:, k * n : (k + 1) * n],
                            start=(k == 0),
                            stop=(k == kd - 1),
                        )
                    ps3 = psh.tile([P, n], F32, tag="h3")
                    for k in range(kd):
                        nc.tensor.matmul(
                            ps3[:],
                            w3b[mi][:, k, :],
                            x_sb[:, k * n : (k + 1) * n],
                            start=(k == 0),
                            stop=(k == kd - 1),
                        )
                    # silu(h1) * h3 via the HW Silu LUT (one ACT + one mul;
                    # accuracy verified identical to sigmoid+muls on HW)
                    sl = cpool.tile([P, n], F32, tag="silu", bufs=2)
                    nc.scalar.activation(sl[:], ps1[:], silu)
                    nc.vector.tensor_mul(h_sb[:, mi, :], sl[:], ps3[:])

                # GEMM2 with split-k emission: the first 3 m-tiles run
                # k=0..md-2 before anything touches h k-tile md-1 (whose
                # SwiGLU finishes last), hiding the ACT/DVE latency tail.
                NPRE = 3
                pre = []
                for m in range(min(NPRE, mo)):
                    psy_t = psy.tile([P, n], F32, tag="y", name=f"ypre{m}")
                    for k in range(md - 1):
                        nc.tensor.matmul(
                            psy_t[:],
                            w2b[m // gm][:, k, (m % gm) * P : (m % gm + 1) * P],
                            h_sb[:, k, :],
                            start=(k == 0),
                            stop=False,
                        )
                    pre.append(psy_t)
                ot4 = None
                for m in range(mo):
                    if m < len(pre):
                        psy_t = pre[m]
                        nc.tensor.matmul(
                            psy_t[:],
                            w2b[m // gm][:, md - 1, (m % gm) * P : (m % gm + 1) * P],
                            h_sb[:, md - 1, :],
                            start=False,
                            stop=True,
                        )
                    else:
                        psy_t = psy.tile([P, n], F32, tag="y")
                        for k in range(md):
                            nc.tensor.matmul(
                                psy_t[:],
                                w2b[m // gm][:, k, (m % gm) * P : (m % gm + 1) * P],
                                h_sb[:, k, :],
                                start=(k == 0),
                                stop=(k == md - 1),
                            )
                    # OGRP m-tiles share one ot tile and one store DMA:
                    # fewer stores keeps the sync engine's ~0.65us/issue
                    # rate off the critical path (drains the kernel tail),
                    # and 4 bufs = a full chunk of cast->store slack.
                    if m % OGRP == 0:
                        ot4 = cpool.tile([P, OGRP * n], BF16, tag="o", bufs=4)
                    j = m % OGRP
                    nc.vector.tensor_copy(ot4[:, j * n : (j + 1) * n], psy_t[:])
                    if m % OGRP == OGRP - 1:
                        nc.sync.dma_start(
                            yh[m // OGRP, :, OGRP * t0 : OGRP * (t0 + n)], ot4[:]
                        )

    nc.compile()
    return nc


def _get_program(ne, dh, di, nt_eff, chunk_key):
    key = (ne, dh, di, nt_eff, chunk_key)
    prog = _prog_cache.get(key)
    if prog is None:
        chunks = [tuple(c) for c in chunk_key]
        prog = _build_program(ne, dh, di // N_CORES, nt_eff, chunks)
        _prog_cache[key] = prog
    return prog


def kernel(x, group_sizes, w1, w2, w3):
    x = np.asarray(x, dtype=np.float32)
    group_sizes = np.asarray(group_sizes)
    w1 = np.asarray(w1, dtype=np.float32)
    w2 = np.asarray(w2, dtype=np.float32)
    w3 = np.asarray(w3, dtype=np.float32)

    ne, dh, di = w1.shape
    di_sh = di // N_CORES
    kd = dh // P
    md = di_sh // P

    chunks, nt_eff = _plan_chunks(group_sizes)
    if nt_eff == 0:
        return np.zeros((0, dh), dtype=np.float32)

    nc = _get_program(ne, dh, di, nt_eff, tuple(chunks))

    # ---- host-side pack / shard ----
    xe = x[:nt_eff]
    # chunk-contiguous x: chunk at column kd*t0 holds [P, kd, n] C-order,
    # i.e. xh[p, kd*t0 + k*n + t] = x[t0+t, k*128+p]
    xh = np.empty((P, kd * nt_eff), dtype=NPBF16)
    for _, t0, n in chunks:
        blk = xe[t0 : t0 + n].astype(NPBF16).reshape(n, kd, P).transpose(2, 1, 0)
        xh[:, kd * t0 : kd * (t0 + n)] = blk.reshape(P, kd * n)
    # w1/w3: [NC, NE, MD, P(part), KD, P(col)]
    # dh = k*128+p ; di = c*di_sh + mi*128 + j
    w1a = np.ascontiguousarray(
        w1.astype(NPBF16)
        .reshape(ne, kd, P, N_CORES, md, P)
        .transpose(3, 0, 4, 2, 1, 5)
    )
    w3a = np.ascontiguousarray(
        w3.astype(NPBF16)
        .reshape(ne, kd, P, N_CORES, md, P)
        .transpose(3, 0, 4, 2, 1, 5)
    )
    # w2: [NC, NE, GH, P(part), MD, W2G]
    # di = c*di_sh + k*128 + p ; dh = g*W2G + j
    gh = dh // W2G
    w2a = np.ascontiguousarray(
        w2.astype(NPBF16)
        .reshape(ne, N_CORES, md, P, gh, W2G)
        .transpose(1, 0, 4, 3, 2, 5)
    )

    in_maps = [
        {"xh": xh, "w1h": w1a[c], "w3h": w3a[c], "w2h": w2a[c]}
        for c in range(N_CORES)
    ]

    global LAST_RESULTS
    res = run_bass_kernel_spmd(
        nc,
        in_maps,
        core_ids=list(range(N_CORES)),
        trace=TRACE,
        trace_cores=TRACE_CORES,
    )
    LAST_RESULTS = res

    # ---- host-side gather + 8-way partial sum ("all-reduce") ----
    # yh[g, p, OGRP*t0 + j*n + t] = y_partial[t0+t, (OGRP*g+j)*128 + p]
    mo = dh // P
    G = mo // OGRP
    acc = np.zeros((nt_eff, dh), dtype=np.float32)
    for c in range(N_CORES):
        yh = res.results[c]["yh"]  # [G, P, OGRP*nt] bf16
        for _, t0, n in chunks:
            blk = yh[:, :, OGRP * t0 : OGRP * (t0 + n)].astype(np.float32)
            # (g, p, j, t) -> (t, g, j, p)
            blk = blk.reshape(G, P, OGRP, n).transpose(3, 0, 2, 1)
            acc[t0 : t0 + n] += blk.reshape(n, dh)
    return np.ascontiguousarray(acc)



# revision 30
# speedup vs baseline: 1.2042x; 1.0063x over previous
"""Mixtral MoE MLP (ragged grouped-GEMM SwiGLU) on 8 Trainium2 NeuronCores.

Sharding: tensor-parallel over the intermediate dim DI. Core c owns the
DI columns [c*DI/8, (c+1)*DI/8) of w1/w3 (and the matching rows of w2)
for ALL experts. Every core processes ALL tokens -> the Bass program is
identical across cores (true SPMD; only the weight data differs per
core). Each core produces a partial y (rank-DI/8 contribution); the
8-way sum ("all-reduce after w2") is done on the host after gather.

DMA schedule: everything on the SP HWDGE ring (a second ring shares the
8 DMA sem lanes and causes cross-ring FIFO coupling -- measured worse).
Weights are prefetched one full EXPERT ahead (emitted at the first
chunk of the previous expert), x one chunk ahead. PE stalls came from
the chain GEMM2-MM -> cast (psum recycle) -> store (ot-slot WAR) ->
DMA-lane predecessor (a weight burst); broken by chunk-deep ot slack,
OGRP-batched stores, and the expert-ahead prefetch smoothing bursts.

Per-core compute: 3 * 2*NT*DH*(DI/8) = ~25.8 GFLOP, bf16 matmuls with
fp32 PSUM accumulation. Measured ~356-365us/core at 2.4 GHz (PE stream
floor 327.7us); runs land ~1.2x slower when the chip is in the P0
2.0 GHz power state (back-to-back executions).

Layouts (host-packed; x/y chunk-contiguous so every DMA is one
contiguous run per partition):
  xh  [128, KD*NT]   xh[p, KD*t0+k*n+t] = x[t0+t, k*128+p]      (bf16)
  w1h [NE, MD, 128, KD, 128]  w1h[e,mi,p,k,j] = w1[e, k*128+p, dlo+mi*128+j]
  w3h same as w1h
  w2h [NE, GH, 128, MD, 512]  w2h[e,g,p,k,j]  = w2[e, dlo+k*128+p, g*512+j]
  yh  [MO/4, 128, 4*NT]  yh[g,p,4*t0+j*n+t] = y_part[t0+t, (4g+j)*128+p]
where dlo = c*DI_SH is this core's DI offset, (t0, n) a chunk.
"""

import os
import sys

import numpy as np
import ml_dtypes

for _p in ("/opt/trn_rl_repo", "/root/.axon_site/_ro/trn_rl_repo"):
    if os.path.isdir(_p) and _p not in sys.path:
        sys.path.append(_p)

import concourse.bass as bass  # noqa: E402
import concourse.bacc as bacc  # noqa: E402
import concourse.tile as tile  # noqa: E402
import concourse.mybir as mybir  # noqa: E402
from concourse.bass_utils import run_bass_kernel_spmd  # noqa: E402


def _ensure_ntff_hook_shim():
    """concourse's trace path imports antenv.axon_hooks, which this image
    lacks; provide a functional stand-in so tracing works (or degrades
    gracefully) instead of raising ImportError."""
    try:
        import antenv.axon_hooks  # noqa: F401
        return
    except Exception:
        pass
    import types

    try:
        import antenv
    except Exception:
        antenv = types.ModuleType("antenv")
        sys.modules["antenv"] = antenv
    mod = types.ModuleType("antenv.axon_hooks")
    state = {"hook": None, "tried": False}

    def set_axon_ntff_profile_hook(h):
        state["hook"] = h

    def get_axon_ntff_profile_hook():
        if state["hook"] is None and not state["tried"]:
            state["tried"] = True
            try:
                from trn_agent_boot.trn_boot import _ntff_profile_via_ctypes

                state["hook"] = _ntff_profile_via_ctypes(
                    "/opt/axon/libaxon_pjrt.so"
                )
            except Exception:
                state["hook"] = None
        return state["hook"]

    mod.set_axon_ntff_profile_hook = set_axon_ntff_profile_hook
    mod.get_axon_ntff_profile_hook = get_axon_ntff_profile_hook
    sys.modules["antenv.axon_hooks"] = mod
    antenv.axon_hooks = mod


_ensure_ntff_hook_shim()

BF16 = mybir.dt.bfloat16
F32 = mybir.dt.float32
NPBF16 = ml_dtypes.bfloat16

N_CORES = 8
P = 128
NMAX = 512  # max matmul moving free dim (one PSUM bank of fp32)

# Knobs for experimentation from test.py
TRACE = False
TRACE_CORES = None
LAST_RESULTS = None

_prog_cache: dict = {}


def _plan_chunks(group_sizes):
    """Split each expert's token range into near-equal chunks of <= NMAX.

    Returns (chunks, nt_eff) where chunks is a list of (expert, t0, n).
    Chunks are emitted grouped by expert, in a schedule order chosen so
    that (a) the first expert is small (short time-to-first-matmul),
    (b) each expert's weight load is covered by a preceding large
    compute window, (c) the last chunk is small (short output drain).
    """
    per_e = {}
    off = 0
    for e, g in enumerate(group_sizes):
        g = int(g)
        if g > 0:
            k = -(-g // NMAX)
            base, rem = divmod(g, k)
            t = off
            cl = []
            for i in range(k):
                n = base + (1 if i < rem else 0)
                cl.append((e, t, n))
                t += n
            per_e[e] = cl
        off += g

    # Biggest experts first: their long compute windows cover successor
    # weight loads; the expert with the smallest final chunk goes last
    # (short output drain before the exit barrier).
    order = sorted(per_e, key=lambda e: -sum(c[2] for c in per_e[e]))
    if len(order) > 1:
        last = min(order, key=lambda e: per_e[e][-1][2])
        order.remove(last)
        order.append(last)

    chunks = [c for e in order for c in per_e[e]]
    return chunks, off


W2G = 512  # w2 blocks carry this many DH columns per DMA
OGRP = 4  # output m-tiles batched per store DMA (sync-engine issue rate)


def _build_program(ne, dh, di_sh, nt, chunks):
    kd = dh // P     # k-tiles for gemm1/3 (contraction over DH)
    md = di_sh // P  # m-tiles for gemm1/3 == k-tiles for gemm2
    mo = dh // P     # m-tiles for gemm2 (output DH)
    gh = dh // W2G   # w2 DMA groups (W2G//P m-tiles per group)
    gm = W2G // P

    nc = bacc.Bacc(
        "TRN2", target_bir_lowering=False, debug=False, num_devices=N_CORES
    )
    # x and y are packed CHUNK-CONTIGUOUS on the host (chunk ci's block
    # starts at column kd*t0 / OGRP*t0): each chunk DMA is then a single
    # contiguous run per partition (16 KB) instead of kd strided ~1 KB
    # runs -> ~16x fewer descriptors, faster + lower-variance startup.
    xh = nc.dram_tensor("xh", [P, kd * nt], BF16, kind="ExternalInput")
    w1h = nc.dram_tensor("w1h", [ne, md, P, kd, P], BF16, kind="ExternalInput")
    w3h = nc.dram_tensor("w3h", [ne, md, P, kd, P], BF16, kind="ExternalInput")
    w2h = nc.dram_tensor("w2h", [ne, gh, P, md, W2G], BF16, kind="ExternalInput")
    yh = nc.dram_tensor("yh", [mo // OGRP, P, OGRP * nt], BF16, kind="ExternalOutput")

    silu = mybir.ActivationFunctionType.Silu

    # Expert schedule: first chunk index of each expert, in chunk order.
    expert_seq = []  # (first_chunk_idx, expert)
    for ci, (e, t0, n) in enumerate(chunks):
        if ci == 0 or chunks[ci - 1][0] != e:
            expert_seq.append((ci, e))
    # At the first chunk of expert k, prefetch weights of expert k+1 —
    # a full expert of DMA lead (>=24us) instead of one chunk.
    next_expert_at = {
        ci: expert_seq[k + 1][1]
        for k, (ci, e) in enumerate(expert_seq)
        if k + 1 < len(expert_seq)
    }

    with tile.TileContext(nc) as tc:
        with (
            tc.tile_pool(name="w1pool", bufs=12) as w1pool,
            tc.tile_pool(name="w3pool", bufs=12) as w3pool,
            tc.tile_pool(name="w2pool", bufs=8) as w2pool,
            tc.tile_pool(name="xpool", bufs=2) as xpool,
            tc.tile_pool(name="hpool", bufs=2) as hpool,
            tc.tile_pool(name="cpool", bufs=4) as cpool,
            tc.tile_pool(name="psh", bufs=2, space="PSUM") as psh,
            tc.tile_pool(name="psy", bufs=4, space="PSUM") as psy,
        ):
            # PE pre-warm: dummy matmuls on a DVE-zeroed scratch tile (DVE
            # memset clears ~3.5us after engine init vs ~7us for the
            # gpsimd path) keep the PE-busy window unbroken until the
            # first real matmul's inputs land, so the HAM clock-gate
            # reaches K=8/8 before real work.
            # 80 warmups bridge the PE from ~7.3us (engine init done)
            # until the first chunk's w1/x k-halves land (~12.5-13us) --
            # HAM flips to 8/8 at ~10.7us, so the real stream starts warm;
            # measured real0_wait=0 on all cores at 125 warmups, i.e. the
            # data arrives before the warmup drain, so fewer is faster.
            warm_sb = cpool.tile([P, P], BF16, tag="warm", bufs=1)
            nc.vector.memset(warm_sb[:], 0.0)
            warm_ps = psy.tile([P, P], F32, tag="y", name="warm_ps")
            for _ in range(80):
                nc.tensor.matmul(warm_ps[:], warm_sb[:], warm_sb[:])

            wtiles = {}

            def emit_weights(e, crit_x=None):
                w1b = [
                    w1pool.tile([P, kd, P], BF16, tag="w1", name=f"w1b{e}_{i}")
                    for i in range(md)
                ]
                w3b = [
                    w3pool.tile([P, kd, P], BF16, tag="w3", name=f"w3b{e}_{i}")
                    for i in range(md)
                ]
                w2b = [
                    w2pool.tile([P, md, W2G], BF16, tag="w2", name=f"w2b{e}_{i}")
                    for i in range(gh)
                ]
                wtiles[e] = (w1b, w3b, w2b)
                if crit_x is not None:
                    # kernel-entry critical path: interleave w1 block 0, x
                    # and w3 block 0 in k-halves. w3b0 must land with the
                    # early pieces: GEMM3-m0 starts ~2.4us after the first
                    # real matmul and a late w3b0 stalls the PE (measured
                    # 2.5us + a HAM re-throttle on slow cores).
                    x_sb, t0, n = crit_x
                    kh = kd // 2
                    for q in range(2):
                        sl = slice(q * kh, (q + 1) * kh)
                        csl = slice(kd * t0 + q * kh * n, kd * t0 + (q + 1) * kh * n)
                        nc.sync.dma_start(w1b[0][:, sl, :], w1h[e, 0, :, sl])
                        nc.sync.dma_start(x_sb[:, q * kh * n : (q + 1) * kh * n], xh[:, csl])
                        nc.sync.dma_start(w3b[0][:, sl, :], w3h[e, 0, :, sl])
                else:
                    nc.sync.dma_start(w1b[0][:], w1h[e, 0])
                    nc.sync.dma_start(w3b[0][:], w3h[e, 0])
                for mi in range(1, md):
                    nc.sync.dma_start(w1b[mi][:], w1h[e, mi])
                    nc.sync.dma_start(w3b[mi][:], w3h[e, mi])
                for g in range(gh):
                    nc.sync.dma_start(w2b[g][:], w2h[e, g])

            def emit_x(ci):
                e, t0, n = chunks[ci]
                x_sb = xpool.tile([P, kd * n], BF16, tag="x", name=f"x{ci}")
                nc.sync.dma_start(x_sb[:], xh[:, kd * t0 : kd * (t0 + n)])
                return x_sb

            # Kernel entry: expert 0's weights + chunk 0's x.
            e0, t00, n0 = chunks[0]
            x0 = xpool.tile([P, kd * n0], BF16, tag="x", name="x0")
            emit_weights(e0, crit_x=(x0, t00, n0))
            x_pending = {0: x0}

            for ci, (e, t0, n) in enumerate(chunks):
                if ci + 1 < len(chunks):
                    x_pending[ci + 1] = emit_x(ci + 1)
                if ci in next_expert_at:
                    emit_weights(next_expert_at[ci])
                x_sb = x_pending.pop(ci)
                w1b, w3b, w2b = wtiles[e]

                h_sb = hpool.tile([P, md, n], BF16, tag="h")
                for mi in range(md):
                    ps1 = psh.tile([P, n], F32, tag="h1")
                    for k in range(kd):
                        nc.tensor.matmul(
                            ps1[:],
                            w1b[mi][:, k, :],
                            x_sb[:, k * n : (k + 1) * n],
                            start=(k == 0),
                            stop=(k == kd - 1),
                        )
                    ps3 = psh.tile([P, n], F32, tag="h3")
                    for k in range(kd):
                        nc.tensor.matmul(
                            ps3[:],
                            w3b[mi][:, k, :],
                            x_sb[:, k * n : (k + 1) * n],
                            start=(k == 0),
                            stop=(k == kd - 1),
                        )
                    # silu(h1) * h3 via the HW Silu LUT (one ACT + one mul;
                    # accuracy verified identical to sigmoid+muls on HW)
                    sl = cpool.tile([P, n], F32, tag="silu", bufs=2)
                    nc.scalar.activation(sl[:], ps1[:], silu)
                    nc.vector.tensor_mul(h_sb[:, mi, :], sl[:], ps3[:])

                # GEMM2 with split-k emission: the first 3 m-tiles run
                # k=0..md-2 before anything touches h k-tile md-1 (whose
                # SwiGLU finishes last), hiding the ACT/DVE latency tail.
                NPRE = 3
                pre = []
                for m in range(min(NPRE, mo)):
                    psy_t = psy.tile([P, n], F32, tag="y", name=f"ypre{m}")
                    for k in range(md - 1):
                        nc.tensor.matmul(
                            psy_t[:],
                            w2b[m // gm][:, k, (m % gm) * P : (m % gm + 1) * P],
                            h_sb[:, k, :],
                            start=(k == 0),
                            stop=False,
                        )
                    pre.append(psy_t)
                ot4 = None
                for m in range(mo):
                    if m < len(pre):
                        psy_t = pre[m]
                        nc.tensor.matmul(
                            psy_t[:],
                            w2b[m // gm][:, md - 1, (m % gm) * P : (m % gm + 1) * P],
                            h_sb[:, md - 1, :],
                            start=False,
                            stop=True,
                        )
                    else:
                        psy_t = psy.tile([P, n], F32, tag="y")
                        for k in range(md):
                            nc.tensor.matmul(
                                psy_t[:],
                                w2b[m // gm][:, k, (m % gm) * P : (m % gm + 1) * P],
                                h_sb[:, k, :],
                                start=(k == 0),
                                stop=(k == md - 1),
                            )
                    # OGRP m-tiles share one ot tile and one store DMA:
                    # fewer stores keeps the sync engine's ~0.65us/issue
                    # rate off the critical path (drains the kernel tail),
                    # and 4 bufs = a full chunk of cast->store slack.
                    if m % OGRP == 0:
                        ot4 = cpool.tile([P, OGRP * n], BF16, tag="o", bufs=4)
                    j = m % OGRP
                    nc.vector.tensor_copy(ot4[:, j * n : (j + 1) * n], psy_t[:])
                    if m % OGRP == OGRP - 1:
                        nc.sync.dma_start(
                            yh[m // OGRP, :, OGRP * t0 : OGRP * (t0 + n)], ot4[:]
                        )

    nc.compile()
    return nc


def _get_program(ne, dh, di, nt_eff, chunk_key):
    key = (ne, dh, di, nt_eff, chunk_key)
    prog = _prog_cache.get(key)
    if prog is None:
        chunks = [tuple(c) for c in chunk_key]
        prog = _build_program(ne, dh, di // N_CORES, nt_eff, chunks)
        _prog_cache[key] = prog
    return prog


def kernel(x, group_sizes, w1, w2, w3):
    x = np.asarray(x, dtype=np.float32)
    group_sizes = np.asarray(group_sizes)
    w1 = np.asarray(w1, dtype=np.float32)
    w2 = np.asarray(w2, dtype=np.float32)
    w3 = np.asarray(w3, dtype=np.float32)

    ne, dh, di = w1.shape
    di_sh = di // N_CORES
    kd = dh // P
    md = di_sh // P

    chunks, nt_eff = _plan_chunks(group_sizes)
    if nt_eff == 0:
        return np.zeros((0, dh), dtype=np.float32)

    nc = _get_program(ne, dh, di, nt_eff, tuple(chunks))

    # ---- host-side pack / shard ----
    xe = x[:nt_eff]
    # chunk-contiguous x: chunk at column kd*t0 holds [P, kd, n] C-order,
    # i.e. xh[p, kd*t0 + k*n + t] = x[t0+t, k*128+p]
    xh = np.empty((P, kd * nt_eff), dtype=NPBF16)
    for _, t0, n in chunks:
        blk = xe[t0 : t0 + n].astype(NPBF16).reshape(n, kd, P).transpose(2, 1, 0)
        xh[:, kd * t0 : kd * (t0 + n)] = blk.reshape(P, kd * n)
    # w1/w3: [NC, NE, MD, P(part), KD, P(col)]
    # dh = k*128+p ; di = c*di_sh + mi*128 + j
    w1a = np.ascontiguousarray(
        w1.astype(NPBF16)
        .reshape(ne, kd, P, N_CORES, md, P)
        .transpose(3, 0, 4, 2, 1, 5)
    )
    w3a = np.ascontiguousarray(
        w3.astype(NPBF16)
        .reshape(ne, kd, P, N_CORES, md, P)
        .transpose(3, 0, 4, 2, 1, 5)
    )
    # w2: [NC, NE, GH, P(part), MD, W2G]
    # di = c*di_sh + k*128 + p ; dh = g*W2G + j
    gh = dh // W2G
    w2a = np.ascontiguousarray(
        w2.astype(NPBF16)
        .reshape(ne, N_CORES, md, P, gh, W2G)
        .transpose(1, 0, 4, 3, 2, 5)
    )

    in_maps = [
        {"xh": xh, "w1h": w1a[c], "w3h": w3a[c], "w2h": w2a[c]}
        for c in range(N_CORES)
    ]

    global LAST_RESULTS
    res = run_bass_kernel_spmd(
        nc,
        in_maps,
        core_ids=list(range(N_CORES)),
        trace=TRACE,
        trace_cores=TRACE_CORES,
    )
    LAST_RESULTS = res

    # ---- host-side gather + 8-way partial sum ("all-reduce") ----
    # yh[g, p, OGRP*t0 + j*n + t] = y_partial[t0+t, (OGRP*g+j)*128 + p]
    mo = dh // P
    G = mo // OGRP
    acc = np.zeros((nt_eff, dh), dtype=np.float32)
    for c in range(N_CORES):
        yh = res.results[c]["yh"]  # [G, P, OGRP*nt] bf16
        for _, t0, n in chunks:
            blk = yh[:, :, OGRP * t0 : OGRP * (t0 + n)].astype(np.float32)
            # (g, p, j, t) -> (t, g, j, p)
            blk = blk.reshape(G, P, OGRP, n).transpose(3, 0, 2, 1)
            acc[t0 : t0 + n] += blk.reshape(n, dh)
    return np.ascontiguousarray(acc)

